# revision 19
# baseline (speedup 1.0000x reference)
"""Trainium2 Bass kernel for LoRALinear: out = x @ W^T + bias + scaling * (x @ A^T) @ B^T.

Problem shapes (hardcoded): x [4, 2048, 4096] f32, weight [4096, 4096] f32,
bias [4096] f32, lora_A [16, 4096] f32, lora_B [4096, 16] f32, scaling = 2.0.

Strategy: pure data-parallel over the 8192 token rows across 8 NeuronCores
(1024 rows each, no collectives). Host-side prep folds the LoRA update into
the weight (W_eff = W + scaling * B @ A — exact in fp32, then one fp16
round) so each core computes one [1024, 4096] x [4096, 4096] GEMM + bias.

The GEMM itself uses ONE LEVEL OF STRASSEN (build_strassen): measured on
this part, the PE's sustained back-to-back N=512 fp16 matmul rate under
8-core load is ~274-281 ns (the chip downclocks from the 2.4 GHz
single-core rate; LDWEIGHTS is fully hidden; psum-group length is free),
so the plain 2048-matmul schedule is PE-stream-bound at ~594 us. Strassen
cuts streamed columns 12.5% (1792 matmuls): the 5 B-side block
combinations are free on the host, the 5 A-side sums and all P->C
recombinations ride the otherwise-idle DVE, and the bias is folded into
the first P->C update of each C block (no C-init copies, no bias
matmuls). All 8 PSUM banks in flight. Measured: ~518-525 us/exec vs
~596-607 us plain (reps-delta, device-resident inputs). Relative error
~6.6e-4 (fp16 operands, fp32 PSUM, fp16 C accumulation).
"""

import numpy as np

import concourse.mybir as mybir
import concourse.tile as tile
from concourse import bacc, bass_utils

N_CORES = 8
B, S, D_IN, D_OUT, R = 4, 2048, 4096, 4096, 16
SCALING = 2.0
M_TOTAL = B * S            # 8192
M_CORE = M_TOTAL // N_CORES  # 1024
P = 128
KO = D_IN // P             # 32 contraction tiles
N_SLICE = 512
N_SLICES = D_OUT // N_SLICE  # 8
M_TILES = M_CORE // P        # 8
F16 = mybir.dt.float16
F32 = mybir.dt.float32


def build_nc(reps: int = 1, col_split: bool = False, out_mode: str = 'dve', dt16=None,
             n_slice: int = N_SLICE, tweak: int = 0):
    """Build and compile the per-core Bass program. reps>1 wraps the whole
    body in a hardware For_i loop (used only for timing runs). tweak adds
    harmless instructions to perturb the BIR hash (busts the NEFF cache
    when A/B-testing compiler-flag changes)."""
    if dt16 is None:
        dt16 = F16
    N_SL = n_slice
    N_SLS = D_OUT // N_SL
    nc = bacc.Bacc("TRN2", target_bir_lowering=False, debug=False,
                   num_devices=N_CORES)

    xT_d = nc.dram_tensor("xT", [D_IN, M_CORE], dt16, kind="ExternalInput")
    wT_d = nc.dram_tensor("wT", [D_IN, D_OUT], dt16, kind="ExternalInput")
    bias_d = nc.dram_tensor("bias", [1, D_OUT], dt16, kind="ExternalInput")
    out_d = nc.dram_tensor("out", [M_CORE, D_OUT], F32, kind="ExternalOutput")

    xT_r = xT_d.ap().rearrange("(ko p) m -> p ko m", p=P)    # [128, 32, 1024]
    wT_r = wT_d.ap().rearrange("(ko p) n -> p ko n", p=P)    # [128, 32, 4096]
    out_r = out_d.ap().rearrange("(mt p) n -> mt p n", p=P)  # [8, 128, 4096]

    with tile.TileContext(nc) as tc:
        with (
            tc.tile_pool(name="xp", bufs=1) as x_pool,
            tc.tile_pool(name="wp", bufs=2) as w_pool,
            tc.tile_pool(name="cst", bufs=1) as c_pool,
            tc.tile_pool(name="op", bufs=4) as o_pool,
            tc.tile_pool(name="ps", bufs=4, space="PSUM") as ps_pool,
        ):
            def body(_i=None):
                x_sb = x_pool.tile([P, KO, M_CORE], dt16)
                for i in range(8):
                    nc.sync.dma_start(
                        x_sb[:, i * 4:(i + 1) * 4, :],
                        xT_r[:, i * 4:(i + 1) * 4, :])
                bias_sb = c_pool.tile([1, D_OUT], dt16)
                nc.sync.dma_start(bias_sb[:], bias_d.ap())
                ones_sb = c_pool.tile([1, M_CORE], dt16)
                nc.any.memset(ones_sb[:], 1.0)

                for n in range(N_SLS):
                    w_sb = w_pool.tile([P, KO, N_SL], dt16)
                    w_chunks = 8 if n == 0 else 4
                    for i in range(w_chunks):
                        cw = KO // w_chunks
                        nc.sync.dma_start(
                            w_sb[:, i * cw:(i + 1) * cw, :],
                            wT_r[:, i * cw:(i + 1) * cw,
                                 n * N_SL:(n + 1) * N_SL])
                    for mt in range(M_TILES):
                        ps = ps_pool.tile([P, N_SL], F32)
                        for k in range(KO):
                            if col_split:
                                # two concurrent M=64 col-group matmuls:
                                # the weight load of one group overlaps the
                                # other group's compute (LDWEIGHTS is
                                # otherwise serial with the matmul stream).
                                for j in range(2):
                                    nc.tensor.matmul(
                                        ps[64 * j:64 * (j + 1), :],
                                        x_sb[:, k, mt * P + 64 * j:
                                             mt * P + 64 * (j + 1)],
                                        w_sb[:, k, :],
                                        start=(k == 0), stop=False,
                                        tile_position=(0, 64 * j))
                            else:
                                nc.tensor.matmul(
                                    ps[:],
                                    x_sb[:, k, mt * P:(mt + 1) * P],
                                    w_sb[:, k, :],
                                    start=(k == 0), stop=False)
                        # bias epilogue: K=1 ones-row x bias slice
                        nc.tensor.matmul(
                            ps[:],
                            ones_sb[:, mt * P:(mt + 1) * P],
                            bias_sb[:, n * N_SL:(n + 1) * N_SL],
                            start=False, stop=True)
                        if out_mode == 'psum_dma':
                            nc.sync.dma_start(
                                out_r[mt, :, n * N_SL:(n + 1) * N_SL],
                                ps[:])
                        else:
                            o_sb = o_pool.tile([P, N_SL], F32)
                            if out_mode == 'dve':
                                nc.vector.tensor_copy(o_sb[:], ps[:])
                            else:
                                nc.any.tensor_copy(o_sb[:], ps[:])
                            nc.sync.dma_start(
                                out_r[mt, :, n * N_SL:(n + 1) * N_SL],
                                o_sb[:])

            if reps == 1:
                body()
            else:
                with tc.For_i(0, reps, 1) as i:
                    body(i)

    nc.compile()
    return nc


def build_strassen(reps: int = 1, ps_bufs: int = 8, c_bufs: int = 6,
                   w_chunks: int = 2):
    """One-level Strassen: per core C = X·Wᵀ with X [1024, 4096] split into
    [512, 2048] blocks → 7 products of [512,2048]x[2048,2048] = 1792 N=512
    matmuls instead of 2048 (−12.5% PE columns). The 5 B-side block
    combinations are precomputed on host (free); the 5 A-side sums run on the
    idle DVE; products accumulate into fp16 C tiles (init'd with bias, so no
    bias epilogue matmuls either). Output is fp16 (host upcasts to fp32).
    """
    KH = KO // 2          # 16 k-tiles per 2048-deep product
    MH = 4                # 4 m-tiles of 128 per 512-row block
    nc = bacc.Bacc("TRN2", target_bir_lowering=False, debug=False,
                   num_devices=N_CORES)

    xT_d = nc.dram_tensor("xT", [D_IN, M_CORE], F16, kind="ExternalInput")
    wS_d = nc.dram_tensor("wS", [7, D_IN // 2, D_OUT // 2], F16,
                          kind="ExternalInput")
    bias_d = nc.dram_tensor("bias", [P, D_OUT], F16, kind="ExternalInput")
    out_d = nc.dram_tensor("out", [M_CORE, D_OUT], F16, kind="ExternalOutput")

    xT_r = xT_d.ap().rearrange("(ko p) m -> p ko m", p=P)      # [128, 32, 1024]
    wS_r = wS_d.ap().rearrange("b (kt p) n -> b p kt n", p=P)  # [7, 128, 16, 2048]
    # out rows: m = br*512 + mt*128 + p
    out_r = out_d.ap().rearrange("(br mt p) n -> br mt p n", p=P, mt=MH)

    # products in issue order: (name, A-op source, wS index, [(Cblk, sign)])
    # A-op source: ('raw', kt_off, m_off) or ('sum', sum_idx)
    # sums: 0:A11+A22 1:A21+A22 2:A11+A12 3:A12-A22 4:A21-A11
    # C blocks: 0:C11 1:C12 2:C21 3:C22
    PRODUCTS = [
        ("P3", ("raw", 0, 0),    2, [(1, 1.0), (3, 1.0)]),    # A11·(B12−B22)
        ("P4", ("raw", KH, 512), 3, [(0, 1.0), (2, 1.0)]),    # A22·(B21−B11)
        ("P2", ("sum", 1),       1, [(2, 1.0), (3, -1.0)]),   # (A21+A22)·B11
        ("P5", ("sum", 2),       4, [(0, -1.0), (1, 1.0)]),   # (A11+A12)·B22
        ("P1", ("sum", 0),       0, [(0, 1.0), (3, 1.0)]),    # (A11+A22)·(B11+B22)
        ("P7", ("sum", 3),       6, [(0, 1.0)]),              # (A12−A22)·(B21+B22)
        ("P6", ("sum", 4),       5, [(3, 1.0)]),              # (A21−A11)·(B11+B12)
    ]
    # last product feeding each C block (in issue order) → DMA point
    LAST = {2: "P2", 1: "P5", 0: "P7", 3: "P6"}
    # C block → (row-block br, col-block bc)
    CPOS = {0: (0, 0), 1: (0, 1), 2: (1, 0), 3: (1, 1)}
    # A-side sums: (orig_idx, x slice A, x slice B, sign for B), ordered so
    # x12's readers run first, then x21's — frees their shared-pool slots
    # for the later sums (avoids a slot-allocation deadlock at bufs=7).
    SUMS = [
        # x_tiles key = (k_offset, m_offset): A11=(0,0) A12=(KH,0)
        # A21=(0,512) A22=(KH,512); t = tileB*sgn + tileA
        (2, (0, 0), (KH, 0), 1.0),      # A11+A12
        (3, (KH, 0), (KH, 512), -1.0),  # A12−A22
        (1, (0, 512), (KH, 512), 1.0),  # A21+A22
        (4, (0, 512), (0, 0), -1.0),    # A21−A11  (= −A11 + A21)
        (0, (0, 0), (KH, 512), 1.0),    # A11+A22
    ]

    add = mybir.AluOpType.add
    mult = mybir.AluOpType.mult

    with tile.TileContext(nc) as tc:
        with (
            tc.tile_pool(name="xa", bufs=7) as xa_pool,
            tc.tile_pool(name="wp", bufs=2) as w_pool,
            tc.tile_pool(name="cp", bufs=c_bufs) as c_pool,
            tc.tile_pool(name="bp", bufs=1) as b_pool,
            tc.tile_pool(name="ps", bufs=ps_bufs, space="PSUM") as ps_pool,
        ):
            def body(_i=None):
                # X blocks as four [P, KH, 512] tiles sharing slots with the
                # five A-sums (x12/x21 die after the sums → slots recycle).
                # Load order: A11, A22 first — they feed the first two
                # products directly; A21/A12 only feed the DVE sums, whose
                # consumers start two products later.
                x_tiles = {}
                for ko_off, m_off in ((0, 0), (KH, 512), (0, 512), (KH, 0)):
                    t = xa_pool.tile([P, KH, 512], F16, tag="xa",
                                     name=f"x_{ko_off}_{m_off}")
                    for ch in range(2):
                        nc.sync.dma_start(
                            t[:, ch * 8:(ch + 1) * 8, :],
                            xT_r[:, ko_off + ch * 8:ko_off + (ch + 1) * 8,
                                 m_off:m_off + 512])
                    x_tiles[(ko_off, m_off)] = t
                bias_sb = b_pool.tile([P, D_OUT], F16)
                nc.sync.dma_start(bias_sb[:], bias_d.ap())

                a_sb = {}
                for idx, (ka, ma), (kb, mb), sgn in SUMS:
                    t = xa_pool.tile([P, KH, 512], F16, tag="xa")
                    # t = (sliceB * sgn) + sliceA
                    nc.vector.scalar_tensor_tensor(
                        t[:], x_tiles[(kb, mb)][:], sgn,
                        x_tiles[(ka, ma)][:], mult, add)
                    a_sb[idx] = t

                def a_slice(src, k, mt):
                    kind = src[0]
                    if kind == "raw":
                        _, ko_off, m_off = src
                        return x_tiles[(ko_off, m_off)][:, k,
                                                        mt * P:(mt + 1) * P]
                    return a_sb[src[1]][:, k, mt * P:(mt + 1) * P]

                for nh in range(2):           # n-half within each col-block
                    c_sb = {}
                    for blk in range(4):
                        c_sb[blk] = c_pool.tile([P, MH, 1024], F16,
                                                name=f"c{blk}", tag="c")
                    for name, asrc, bi, contribs in PRODUCTS:
                        # P3/P4 are the first contributors to every C block:
                        # their RMW reads the bias tile instead of C (no
                        # C-init copies needed at all).
                        first = name in ("P3", "P4")
                        for ns in range(2):   # 512-slice within the n-half
                            col = nh * 1024 + ns * 512
                            w_sb = w_pool.tile([P, KH, N_SLICE], F16)
                            for ch in range(w_chunks):
                                cw = KH // w_chunks
                                nc.sync.dma_start(
                                    w_sb[:, ch * cw:(ch + 1) * cw, :],
                                    wS_r[bi, :, ch * cw:(ch + 1) * cw,
                                         col:col + N_SLICE])
                            for mt in range(MH):
                                ps = ps_pool.tile([P, N_SLICE], F32)
                                for k in range(KH):
                                    nc.tensor.matmul(
                                        ps[:], a_slice(asrc, k, mt),
                                        w_sb[:, k, :],
                                        start=(k == 0), stop=(k == KH - 1))
                                for blk, sgn in contribs:
                                    csl = c_sb[blk][:, mt,
                                                    ns * N_SLICE:(ns + 1) * N_SLICE]
                                    br, bc = CPOS[blk]
                                    gcol = bc * 2048 + col
                                    src1 = (bias_sb[:, gcol:gcol + N_SLICE]
                                            if first else csl)
                                    nc.vector.scalar_tensor_tensor(
                                        csl, ps[:], sgn, src1, mult, add)
                        for blk, last in LAST.items():
                            if last == name:
                                br, bc = CPOS[blk]
                                col = bc * 2048 + nh * 1024
                                for mt in range(MH):
                                    nc.sync.dma_start(
                                        out_r[br, mt, :, col:col + 1024],
                                        c_sb[blk][:, mt, :])

            if reps == 1:
                body()
            else:
                with tc.For_i(0, reps, 1) as i:
                    body(i)

    nc.compile()
    return nc


def prep_in_maps_strassen(x, weight, bias, lora_A, lora_B):
    xf = np.asarray(x, dtype=np.float32).reshape(M_TOTAL, D_IN)
    w_eff = np.asarray(weight, dtype=np.float32) + SCALING * (
        np.asarray(lora_B, dtype=np.float32) @ np.asarray(lora_A, dtype=np.float32))
    WT = np.ascontiguousarray(w_eff.T)           # [K=4096, N=4096] fp32
    H = D_IN // 2
    B11, B12 = WT[:H, :H], WT[:H, H:]
    B21, B22 = WT[H:, :H], WT[H:, H:]
    combos = [B11 + B22, B11, B12 - B22, B21 - B11, B22, B11 + B12, B21 + B22]
    wS = np.stack([c.astype(np.float16) for c in combos], axis=0)
    bias128 = np.tile(np.asarray(bias, dtype=np.float32
                                 ).astype(np.float16).reshape(1, D_OUT), (P, 1))
    in_maps = []
    for c in range(N_CORES):
        xT_c = np.ascontiguousarray(
            xf[c * M_CORE:(c + 1) * M_CORE].T).astype(np.float16)
        in_maps.append({"xT": xT_c, "wS": wS, "bias": bias128})
    return in_maps


_NC_CACHE = {}


def _get_nc(reps: int = 1, col_split: bool = False, out_mode: str = 'dve', dt16=None,
            n_slice: int = N_SLICE, strassen: bool = False, ps_bufs: int = 8,
            c_bufs: int = 6, w_chunks: int = 2):
    if strassen:
        key = ('strassen', reps, ps_bufs, c_bufs, w_chunks)
        if key not in _NC_CACHE:
            _NC_CACHE[key] = build_strassen(reps, ps_bufs, c_bufs, w_chunks)
        return _NC_CACHE[key]
    key = (reps, col_split, out_mode, str(dt16), n_slice)
    if key not in _NC_CACHE:
        _NC_CACHE[key] = build_nc(reps, col_split, out_mode, dt16, n_slice)
    return _NC_CACHE[key]


def prep_in_maps(x, weight, bias, lora_A, lora_B):
    """Host-side shard + pack: returns in_maps for the 8 cores."""
    xf = np.asarray(x, dtype=np.float32).reshape(M_TOTAL, D_IN)
    w_eff = np.asarray(weight, dtype=np.float32) + SCALING * (
        np.asarray(lora_B, dtype=np.float32) @ np.asarray(lora_A, dtype=np.float32))
    wT = np.ascontiguousarray(w_eff.T).astype(np.float16)
    bias1 = np.asarray(bias, dtype=np.float32).astype(np.float16).reshape(1, D_OUT)
    in_maps = []
    for c in range(N_CORES):
        xT_c = np.ascontiguousarray(
            xf[c * M_CORE:(c + 1) * M_CORE].T).astype(np.float16)
        in_maps.append({"xT": xT_c, "wT": wT, "bias": bias1})
    return in_maps


STRATEGY = "strassen"   # "strassen" or "plain"


def kernel(x, weight, bias, lora_A, lora_B):
    if STRATEGY == "strassen":
        nc = _get_nc(1, strassen=True)
        in_maps = prep_in_maps_strassen(x, weight, bias, lora_A, lora_B)
        res = bass_utils.run_bass_kernel_spmd(nc, in_maps,
                                              core_ids=list(range(N_CORES)))
        out = np.concatenate(
            [res.results[c]["out"].astype(np.float32) for c in range(N_CORES)],
            axis=0)
        return out.reshape(B, S, D_OUT)
    nc = _get_nc(1)
    in_maps = prep_in_maps(x, weight, bias, lora_A, lora_B)
    res = bass_utils.run_bass_kernel_spmd(nc, in_maps, core_ids=list(range(N_CORES)))
    out = np.concatenate([res.results[c]["out"] for c in range(N_CORES)], axis=0)
    return out.reshape(B, S, D_OUT)

